# revision 8
# baseline (speedup 1.0000x reference)
"""LIFSpike Trainium2 kernel (Bass/Tile), SPMD over 8 NeuronCores.

Reference semantics (T=4, tau=2, vth=1, vreset=0, decay_input=False,
hard reset):
    xs = x.reshape(T, B//T, C, H, W)
    v0 = 0
    h_t = v_t * 0.5 + x_t
    s_t = (h_t >= 1.0)
    v_{t+1} = h_t * (h_t < 1.0)
    out = s.reshape(B, C, H, W)

Kernel-side reformulation (exact in fp32 -- all rescalings are by powers
of two, which commute with fp rounding):
    r_t := 2^t * h_t,  host supplies x'_t = 2^t * x_t
    r_0     = x'_0                                   (plain DMA load)
    s_t     = (r_t >= 2^t)
    q_t     = (r_t < 2^t) * r_t   (= 2^t * v_{t+1})  (one STT op, DVE)
    r_{t+1} = q_t + x'_{t+1}

Engine assignment (chosen to avoid the DVE<->Pool SBUF-port contention
measured at ~3x mutual slowdown):
  * r_{t+1} adds run on the otherwise-idle Tensor engine as an identity
    matmul accumulation group:  psum = I @ q_t ; psum += I @ x'_{t+1}.
    This is exact in fp32: products are 1.0*v and 0.0*v, and PSUM
    accumulates in fp32.
  * The spike comparison runs on the Scalar (ACT) engine, reading PSUM
    directly:  sign(r_t + bias_t), bias_t = nextafter(-2^t, 0), fp8 out;
    the host decodes s = (value > 0).  Exact: r + bias == 0 only for
    r == 2^t*(1-2^-24) (the largest f32 below threshold), and sign(0)=0
    decodes to s=0, which is correct.
  * The reset STT runs on DVE from an SBUF copy of r (copies split
    between DVE and ACT).
  * All 16 MiB of input streams through plain HWDGE loads with no
    compute dependency, so DMA -- the roofline resource -- never stalls.

Host-side input layout per core (partition-major, t-major):
    x_core[p, t*8192 + b*2048 + j] = 2^t * x[t*32 + core*4 + b, flat=p*2048+j]
Output layout is b-major:
    s_core[p, b*8192 + t*2048 + j]
"""

import numpy as np

T = 4
BP = 32               # B // T
NCORES = 8
BPC = BP // NCORES    # chains per core = 4
SLICE = 256 * 32 * 32  # elements per (t, b) slice = 262144
P = 128
W = SLICE // P        # free elems per chain-timestep tile = 2048
FREE_T = BPC * W      # 8192 (one timestep slab, all chains)
FREE = T * FREE_T     # 32768
MMW = 512             # fp32 moving-operand max free dim per matmul

_cache = {}


def _build_program():
    import concourse.bass as bass
    import concourse.tile as tile
    from concourse import bacc, mybir

    Alu = mybir.AluOpType
    Act = mybir.ActivationFunctionType
    f32 = mybir.dt.float32
    out_dt = mybir.dt.float8e4

    nc = bacc.Bacc(debug=False)
    x = nc.dram_tensor("x", [P, FREE], f32, kind="ExternalInput").ap()
    eye = nc.dram_tensor("eye", [P, P], f32, kind="ExternalInput").ap()
    s = nc.dram_tensor("s", [P, FREE], out_dt, kind="ExternalOutput").ap()

    with tile.TileContext(nc) as tc:
        with (
            tc.tile_pool(name="state", bufs=1) as vpool,
            tc.tile_pool(name="sout", bufs=1) as spool,
            tc.psum_pool(name="acc", bufs=2) as ppool,
        ):
            slabs = [
                vpool.tile([P, FREE_T], f32, tag=f"xs{t}", name=f"xs{t}")
                for t in range(T)
            ]
            qts = [
                vpool.tile([P, W], f32, tag=f"q{b}", name=f"q{b}")
                for b in range(BPC)
            ]
            outs = [
                spool.tile([P, T * W], out_dt, tag=f"s{b}", name=f"sout{b}")
                for b in range(BPC)
            ]
            eye_t = vpool.tile([P, P], f32, tag="eye", name="eye_t")
            biases = vpool.tile([P, T], f32, tag="bias", name="bias")
            for t in range(T):
                bias = float(np.nextafter(np.float32(-(1 << t)), np.float32(0)))
                nc.gpsimd.memset(biases[:, t:t + 1], bias)

            nc.sync.dma_start(eye_t[:], eye[:, :])
            # stream all input up front; t0 split for an early pipeline start
            nc.sync.dma_start(slabs[0][:, :2 * W], x[:, :2 * W])
            nc.sync.dma_start(slabs[0][:, 2 * W:], x[:, 2 * W:FREE_T])
            for t in range(1, T):
                nc.sync.dma_start(
                    slabs[t][:], x[:, t * FREE_T:(t + 1) * FREE_T]
                )

            for t in range(T):
                th = float(1 << t)
                for b in range(BPC):
                    p = slabs[t][:, b * W:(b + 1) * W]
                    if t == 0:
                        # spike from SBUF state
                        nc.scalar.activation(
                            outs[b][:, :W], p, Act.Sign, bias=biases[:, :1]
                        )
                    if t < T - 1:
                        q = qts[b]
                        nc.vector.scalar_tensor_tensor(
                            q[:], p, th, p, Alu.is_lt, Alu.mult
                        )
                        # r_{t+1} = q + x'_{t+1} on the Tensor engine
                        acc = ppool.tile(
                            [P, W], f32, tag="acc", name=f"acc{b}_{t}"
                        )
                        nxt = slabs[t + 1][:, b * W:(b + 1) * W]
                        for c in range(0, W, MMW):
                            nc.tensor.matmul(
                                acc[:, c:c + MMW], eye_t[:], q[:, c:c + MMW],
                                start=True, stop=False,
                            )
                            nc.tensor.matmul(
                                acc[:, c:c + MMW], eye_t[:],
                                nxt[:, c:c + MMW],
                                start=False, stop=True,
                            )
                        # spike for t+1 straight from PSUM (ACT)
                        nc.scalar.activation(
                            outs[b][:, (t + 1) * W:(t + 2) * W], acc[:],
                            Act.Sign, bias=biases[:, t + 1:t + 2],
                        )
                        if t + 1 < T - 1:
                            # SBUF copy of r_{t+1} for the next STT; split
                            # between DVE and ACT to balance the engines
                            if b % 2 == 0:
                                nc.vector.tensor_copy(nxt, acc[:])
                            else:
                                nc.scalar.activation(
                                    nxt, acc[:], Act.Copy, bias=0.0
                                )
                    if t == 0:
                        # first-half store rides the scalar HWDGE ring
                        pass
                    if t == 1:
                        nc.scalar.dma_start(
                            s[:, b * T * W:b * T * W + 2 * W],
                            outs[b][:, :2 * W],
                        )
                    if t == T - 1:
                        nc.scalar.dma_start(
                            s[:, b * T * W + 2 * W:(b + 1) * T * W],
                            outs[b][:, 2 * W:],
                        )
    nc.compile()
    return nc


def _shard(x):
    # x: (128, 256, 32, 32) f32 -> list of 8 per-core [128, 32768] arrays,
    # timestep t pre-scaled by 2^t (exact in fp32)
    xr = np.ascontiguousarray(x).reshape(T, BP, SLICE)
    tscale = (2.0 ** np.arange(T, dtype=np.float32)).astype(np.float32)
    shards = []
    for k in range(NCORES):
        xk = xr[:, k * BPC:(k + 1) * BPC, :].reshape(T, BPC, P, W)
        xk = xk * tscale[:, None, None, None]
        xk = xk.transpose(2, 0, 1, 3).reshape(P, FREE)
        shards.append(np.asarray(xk, dtype=np.float32))
    return shards


def _unshard(parts):
    # parts: 8 per-core [128, 32768] arrays (fp8 sign values, b-major)
    # -> (128,256,32,32) f32 spikes; spike iff stored value > 0
    out = np.empty((T, BP, SLICE), dtype=np.float32)
    for k, sk in enumerate(parts):
        sk = (np.asarray(sk).astype(np.float32) > 0).astype(np.float32)
        sk = sk.reshape(P, BPC, T, W)
        out[:, k * BPC:(k + 1) * BPC, :] = (
            sk.transpose(2, 1, 0, 3).reshape(T, BPC, SLICE)
        )
    return out.reshape(T * BP, 256, 32, 32)


def _in_maps(x):
    eye_np = np.eye(P, dtype=np.float32)
    return [
        {"x": sk, "eye": eye_np}
        for sk in _shard(np.asarray(x, dtype=np.float32))
    ]


def kernel(x):
    from concourse.bass_utils import run_bass_kernel_spmd

    if "nc" not in _cache:
        _cache["nc"] = _build_program()
    nc = _cache["nc"]

    res = run_bass_kernel_spmd(nc, _in_maps(x), list(range(NCORES)))
    return _unshard([res.results[k]["s"] for k in range(NCORES)])


# revision 10
# speedup vs baseline: 1.2946x; 1.2946x over previous
"""LIFSpike Trainium2 kernel (Bass/Tile), SPMD over 8 NeuronCores.

Reference semantics (T=4, tau=2, vth=1, vreset=0, decay_input=False,
hard reset):
    xs = x.reshape(T, B//T, C, H, W)
    v0 = 0
    h_t = v_t * 0.5 + x_t
    s_t = (h_t >= 1.0)
    v_{t+1} = h_t * (h_t < 1.0)
    out = s.reshape(B, C, H, W)

Kernel-side reformulation (exact in fp32 -- all rescalings are by powers
of two, which commute with fp rounding):
    r_t := 2^t * h_t,  host supplies x'_t = 2^t * x_t
    r_0     = x'_0                                   (plain DMA load)
    s_t     = (r_t >= 2^t)
    q_t     = (r_t < 2^t) * r_t   (= 2^t * v_{t+1})  (one STT op, DVE)
    r_{t+1} = q_t + x'_{t+1}

Engine assignment (DVE and the Pool engine contend ~3x on shared SBUF
ports, and fp32 matmul adds on PE measured 2-pass/too slow, so):
  * STT and the t1/t3 adds run on DVE (phase-ordered for pipelining).
  * The t2 adds ride on SWDGE accumulate-DMAs: the STT writes q into
    the t2 slab slice, then the DMA adds x'_2 from HBM on top.
  * The spike comparison runs on the Scalar (ACT) engine:
    sign(r_t + bias_t), bias_t = nextafter(-2^t, 0), fp8 out; the host
    decodes s = (value > 0).  Exact: r + bias == 0 only for
    r == 2^t*(1-2^-24), and sign(0)=0 decodes to s=0, correct.
  * Plain loads round-robin over BOTH HWDGE rings (sync + scalar):
    one ring sustains only ~312 GB/s; two together ~380+.

Host-side input layout per core (partition-major, t-major):
    x_core[p, t*8192 + b*2048 + j] = 2^t * x[t*32 + core*4 + b, flat=p*2048+j]
Output layout is b-major:
    s_core[p, b*8192 + t*2048 + j]
"""

import numpy as np

T = 4
BP = 32               # B // T
NCORES = 8
BPC = BP // NCORES    # chains per core = 4
SLICE = 256 * 32 * 32  # elements per (t, b) slice = 262144
P = 128
W = SLICE // P        # free elems per chain-timestep tile = 2048
FREE_T = BPC * W      # 8192 (one timestep slab, all chains)
FREE = T * FREE_T     # 32768
ACCUM_T = 2           # timestep whose x-add rides on accumulate-DMAs

_cache = {}


def _build_program():
    import concourse.bass as bass
    import concourse.tile as tile
    from concourse import bacc, mybir

    Alu = mybir.AluOpType
    Act = mybir.ActivationFunctionType
    f32 = mybir.dt.float32
    out_dt = mybir.dt.float8e4

    nc = bacc.Bacc(debug=False)
    x = nc.dram_tensor("x", [P, FREE], f32, kind="ExternalInput").ap()
    s = nc.dram_tensor("s", [P, FREE], out_dt, kind="ExternalOutput").ap()

    with tile.TileContext(nc) as tc:
        with (
            tc.tile_pool(name="state", bufs=1) as vpool,
            tc.tile_pool(name="sout", bufs=1) as spool,
        ):
            slabs = [
                vpool.tile([P, FREE_T], f32, tag=f"xs{t}", name=f"xs{t}")
                for t in range(T)
            ]
            qts = [
                vpool.tile([P, W], f32, tag=f"q{b}", name=f"q{b}")
                for b in range(BPC)
            ]
            outs = [
                spool.tile([P, T * W], out_dt, tag=f"s{b}", name=f"sout{b}")
                for b in range(BPC)
            ]
            biases = vpool.tile([P, T], f32, tag="bias", name="bias")
            for t in range(T):
                bias = float(np.nextafter(np.float32(-(1 << t)), np.float32(0)))
                nc.gpsimd.memset(biases[:, t:t + 1], bias)

            # plain loads for every timestep except ACCUM_T, 1 MiB chunks,
            # round-robin over the two HWDGE rings
            rings = [nc.sync, nc.scalar]
            ring_i = 0
            for t in range(T):
                if t == ACCUM_T:
                    continue
                for b in range(BPC):
                    lo = t * FREE_T + b * W
                    rings[ring_i % 2].dma_start(
                        slabs[t][:, b * W:(b + 1) * W], x[:, lo:lo + W]
                    )
                    ring_i += 1

            def state(b, t):
                return slabs[t][:, b * W:(b + 1) * W]

            for t in range(T):
                th = float(1 << t)
                # spikes for this timestep (ACT queue)
                for b in range(BPC):
                    nc.scalar.activation(
                        outs[b][:, t * W:(t + 1) * W], state(b, t), Act.Sign,
                        bias=biases[:, t:t + 1],
                    )
                # output stores as soon as a chain's half is complete
                if t == 1:
                    for b in range(BPC):
                        rings[b % 2].dma_start(
                            s[:, b * T * W:b * T * W + 2 * W],
                            outs[b][:, :2 * W],
                        )
                if t == T - 1:
                    for b in range(BPC):
                        rings[b % 2].dma_start(
                            s[:, b * T * W + 2 * W:(b + 1) * T * W],
                            outs[b][:, 2 * W:],
                        )
                    continue
                # phase 1: all STTs for this timestep (DVE FIFO stays fed)
                for b in range(BPC):
                    p = state(b, t)
                    if t + 1 == ACCUM_T:
                        # q lands in the t2 slab slice; the accum-DMA adds
                        # x'_2 from HBM on top (SWDGE, Pool ring)
                        nxt = state(b, t + 1)
                        nc.vector.scalar_tensor_tensor(
                            nxt, p, th, p, Alu.is_lt, Alu.mult
                        )
                        lo = (t + 1) * FREE_T + b * W
                        nc.gpsimd.dma_start(
                            nxt, x[:, lo:lo + W], accum_op=Alu.add
                        )
                    else:
                        nc.vector.scalar_tensor_tensor(
                            qts[b][:], p, th, p, Alu.is_lt, Alu.mult
                        )
                # phase 2: the r_{t+1} adds (skipped for the accum timestep)
                if t + 1 != ACCUM_T:
                    for b in range(BPC):
                        nxt = state(b, t + 1)
                        nc.vector.tensor_tensor(nxt, nxt, qts[b][:], Alu.add)
    nc.compile()
    return nc


def _shard(x):
    # x: (128, 256, 32, 32) f32 -> list of 8 per-core [128, 32768] arrays,
    # timestep t pre-scaled by 2^t (exact in fp32)
    xr = np.ascontiguousarray(x).reshape(T, BP, SLICE)
    tscale = (2.0 ** np.arange(T, dtype=np.float32)).astype(np.float32)
    shards = []
    for k in range(NCORES):
        xk = xr[:, k * BPC:(k + 1) * BPC, :].reshape(T, BPC, P, W)
        xk = xk * tscale[:, None, None, None]
        xk = xk.transpose(2, 0, 1, 3).reshape(P, FREE)
        shards.append(np.asarray(xk, dtype=np.float32))
    return shards


def _unshard(parts):
    # parts: 8 per-core [128, 32768] arrays (fp8 sign values, b-major)
    # -> (128,256,32,32) f32 spikes; spike iff stored value > 0
    out = np.empty((T, BP, SLICE), dtype=np.float32)
    for k, sk in enumerate(parts):
        sk = (np.asarray(sk).astype(np.float32) > 0).astype(np.float32)
        sk = sk.reshape(P, BPC, T, W)
        out[:, k * BPC:(k + 1) * BPC, :] = (
            sk.transpose(2, 1, 0, 3).reshape(T, BPC, SLICE)
        )
    return out.reshape(T * BP, 256, 32, 32)


def _in_maps(x):
    return [{"x": sk} for sk in _shard(np.asarray(x, dtype=np.float32))]


def kernel(x):
    from concourse.bass_utils import run_bass_kernel_spmd

    if "nc" not in _cache:
        _cache["nc"] = _build_program()
    nc = _cache["nc"]

    res = run_bass_kernel_spmd(nc, _in_maps(x), list(range(NCORES)))
    return _unshard([res.results[k]["s"] for k in range(NCORES)])
